# revision 21
# baseline (speedup 1.0000x reference)
"""Trainium2 Bass kernel for nn_D3Net (moe_routing).

Data-parallel over 8 NeuronCores: core c takes samples [c, c+8, c+16, c+24].
Per sample: decision network -> gate; ADB branch (2x gnconv with depthwise
7x7); gated residual combine.

Layout: channels on SBUF partitions, pixels (y-major) on the free dim.
Dense convs/matmuls in bf16 (PE, 1 cyc/row); LN statistics in fp32r.
Depthwise 7x7 = 49 scalar_tensor_tensor MACs per channel group:
low group (abc 0..119, partitions 8..127) on VectorE, high group
(abc 120..247) on GpSimd, in bf16.

`kernel(**inputs)` is the host entry: preps weight layouts, compiles once,
runs SPMD on cores 0-7 via run_bass_kernel_spmd, reassembles full output.
"""

import sys

sys.path.insert(0, "/opt/trn_rl_repo")

import numpy as np

N_CORES = 8
B, C, H, W = 32, 128, 64, 64
PIX = H * W            # 4096
SPB = B // N_CORES     # 4 samples per core

# padded abc layout for depthwise 7x7: 70 rows x 72 cols, interior at (3, 3)
DW_W = 72
DW_F = 70 * DW_W       # 5040
DW_BASE = 3 * DW_W + 3

# dn stages: (in_ch, out_ch, in_side); padded input tile: (side+2) x (side+2)
DN_STAGES = [(128, 64, 64), (64, 32, 32), (32, 16, 16), (16, 8, 8)]

FB = 512    # f-chunk for matmul moving operands (ISA limit)
NB = PIX // FB  # 8
FS = 512    # f-chunk for fp32r LN statistic matmuls
NS = PIX // FS  # 8
COMP_K = [8, 128, 128]


def build_nc():
    from contextlib import ExitStack

    import concourse.bacc as bacc
    import concourse.mybir as mybir
    from concourse import tile

    dt = mybir.dt
    Alu = mybir.AluOpType
    Act = mybir.ActivationFunctionType
    f32r = dt.float32r

    nc = bacc.Bacc("TRN2", target_bir_lowering=False, debug=False,
                   num_devices=N_CORES)

    def din(name, shape, d=dt.float32):
        return nc.dram_tensor(name, shape, d, kind="ExternalInput")

    x_in = din("x_in", (SPB, C, PIX))
    rp_in = din("rp_in", (SPB, C, PIX), dt.bfloat16)
    cp_in = din("cp_in", (SPB, C, PIX), dt.bfloat16)
    out_d = nc.dram_tensor("out", (SPB, C, PIX), dt.float32, kind="ExternalOutput")

    dn_w = [din(f"dn_w{i}", (ci, 9 * co), dt.bfloat16) for i, (ci, co, _) in enumerate(DN_STAGES)]
    dn_b = [din(f"dn_b{i}", (co, 1)) for i, (_, co, _) in enumerate(DN_STAGES)]
    dn4_wd = din("dn4_wd", (8, 1), dt.bfloat16)
    thr_in = din("thr", (1, SPB))        # thr - bd, pre-adjusted on host
    rs_in = din("rs", (1, 1))            # res_scale
    ln1_gb = din("ln1_gb", (C, 2))
    eps_in = din("eps", (8, 1))
    ones_bf = din("ones_bf", (C, 8), dt.bfloat16)
    onehot = din("onehot", (8, C), dt.bfloat16)  # bcast lhsT: row-0 selector
    pi_w = din("pi_w", (C, 8 * C), dt.bfloat16)   # [(ci%128), (br, kg, mg, co)]
    pi_b = din("pi_b", (C, 4))                    # [(co%128), (br, mg)]
    dww = din("dww", (C, 4 * 49))                 # [(part), (br, grp, tap)]
    dwb0 = din("dwb0", (C, 2))                    # rows 8:16 = dw bias 0:8
    comp_w = din("comp_w", (C, 6 * C), dt.bfloat16)   # [(k), (br, term{y1,lo,hi}, co)]
    pwa_w = din("pwa_w", (C, 2 * 2 * 8), dt.bfloat16)  # [(ci), (br, kg, 8)]
    pwab = din("pwab", (8, 2))                         # pwa bias [8, br]
    bconst = din("bconst", (C, 2))
    dwdiag = din("dwdiag", (C, 98 * C), dt.bfloat16)  # PE-tap diags, packed
    beta_g = din("beta_g", (C, 2))
    c11_wT = din("c11_wT", (C, C), dt.bfloat16)
    c11_b = din("c11_b", (C, 1))

    ctx = ExitStack()
    with tile.TileContext(nc, pool_alloc_mode="queue") as tc:
        wp = ctx.enter_context(tc.tile_pool(name="wpool", bufs=1))
        pp = ctx.enter_context(tc.tile_pool(name="perm", bufs=1))
        dpool = ctx.enter_context(tc.tile_pool(name="data", bufs=1))
        db2 = ctx.enter_context(tc.tile_pool(name="dbuf", bufs=2))
        ps = ctx.enter_context(tc.tile_pool(name="psum", bufs=2, space="PSUM"))
        ps4 = ctx.enter_context(tc.tile_pool(name="psum4", bufs=2, space="PSUM"))
        ps1 = ctx.enter_context(tc.tile_pool(name="psum1", bufs=1, space="PSUM"))

        def wtile(src, shape):
            t = wp.tile(list(shape),
                        src.tensor.dtype if hasattr(src, "tensor") else src.dtype,
                        tag=f"w_{src.name}")
            nc.sync.dma_start(out=t[:], in_=src.ap())
            return t

        w_dn = [wtile(dn_w[i], (DN_STAGES[i][0], 9 * DN_STAGES[i][1])) for i in range(4)]
        b_dn = [wtile(dn_b[i], (DN_STAGES[i][1], 1)) for i in range(4)]
        w_d4 = wtile(dn4_wd, (8, 1))
        t_thr = wtile(thr_in, (1, SPB))
        t_rs = wtile(rs_in, (1, 1))
        t_ln1 = wtile(ln1_gb, (C, 2))
        t_eps = wtile(eps_in, (8, 1))
        t_onb = wtile(ones_bf, (C, 8))
        t_oh = wtile(onehot, (8, C))
        t_piw = wtile(pi_w, (C, 8 * C))
        t_pib = wtile(pi_b, (C, 4))
        t_dww = wtile(dww, (C, 4 * 49))
        t_dwb0 = wtile(dwb0, (C, 2))
        t_comp = wtile(comp_w, (C, 6 * C))
        t_pwaw = wtile(pwa_w, (C, 2 * 2 * 8))
        t_pwab = wtile(pwab, (8, 2))
        t_bc = wtile(bconst, (C, 2))
        t_bg = wtile(beta_g, (C, 2))
        t_c11 = wtile(c11_wT, (C, C))
        t_c11b = wtile(c11_b, (C, 1))

        # persistent padded tiles; zero once, interiors rewritten per sample
        dn_pads = []
        for si, (ci, _, side) in enumerate(DN_STAGES):
            t = pp.tile([ci, (side + 2) * (side + 2)], dt.bfloat16, tag=f"dnpad{si}")
            nc.vector.memset(t[:], 0.0)
            dn_pads.append(t)
        flo_p = pp.tile([C, DW_F], dt.bfloat16, tag="flo")
        fhi_p = pp.tile([C, DW_F], dt.bfloat16, tag="fhi")
        nc.vector.memset(flo_p[:], 0.0)
        nc.vector.memset(fhi_p[:], 0.0)

        def win(ap_2d, rows, stride, r0, nr, c0, ncol):
            """3D window view of a flat [P, rows*stride] AP."""
            v = ap_2d.rearrange("p (r c) -> p r c", r=rows, c=stride)
            return v[:, r0:r0 + nr, c0:c0 + ncol]

        def ln_norm(src, src_is_f32, out_t, gb):
            """out = (src - mu(ch)) * rstd(ch) [* g + b]; src [C, PIX] tile.

            Per 512-pixel chunk: matmul with [C, 8] ones/C -> psum [8, FS]
            (8 identical rows, base partition 0), compact rsqrt via
            exp(-0.5*ln(var+eps)), broadcast back to 128 partitions via a
            one-hot-row matmul, then normalize with two DVE passes.
            """
            xsq = dpool.tile([C, PIX], dt.bfloat16, tag="scratch")
            nc.scalar.square(xsq[:], src[:])
            if src_is_f32:
                src_bf = dpool.tile([C, PIX], dt.bfloat16, tag="srcbf")
                nc.vector.tensor_copy(src_bf[:], src[:])
            else:
                src_bf = src
            for j in range(NS):
                sl = slice(j * FS, (j + 1) * FS)
                stats = ps1.tile([8, 2 * FS], dt.float32, tag="auxs")
                nc.tensor.matmul(stats[:, 0:FS], t_onb[:],
                                 src_bf[:, sl], start=True, stop=True)
                nc.tensor.matmul(stats[:, FS:2 * FS], t_onb[:],
                                 xsq[:, sl], start=True, stop=True)
                musq = dpool.tile([8, FS], dt.float32, tag="musq")
                nc.scalar.square(musq[:], stats[:, 0:FS])
                var_c = dpool.tile([8, FS], dt.float32, tag="varc")
                nc.vector.scalar_tensor_tensor(var_c[:], musq[:], -1.0,
                                               stats[:, FS:2 * FS], Alu.mult, Alu.add)
                lnv = dpool.tile([8, FS], dt.float32, tag="lnv")
                nc.scalar.activation(lnv[:], var_c[:], Act.Ln, bias=t_eps[:])
                rstd_c = dpool.tile([8, FS], dt.bfloat16, tag="rstdc")
                nc.scalar.activation(rstd_c[:], lnv[:], Act.Exp, scale=-0.5)
                murstd_c = dpool.tile([8, FS], dt.bfloat16, tag="murstdc")
                nc.vector.tensor_mul(murstd_c[:], stats[:, 0:FS], rstd_c[:])
                bco = ps1.tile([C, FS], dt.float32, tag="auxb")
                nc.tensor.matmul(bco[:], t_oh[:], rstd_c[:], start=True, stop=True)
                tmp = dpool.tile([C, FS], dt.float32, tag="lntmp")
                nc.vector.tensor_mul(tmp[:], src[:, sl], bco[:])
                bco2 = ps1.tile([C, FS], dt.float32, tag="auxb")
                nc.tensor.matmul(bco2[:], t_oh[:], murstd_c[:], start=True, stop=True)
                if gb is None:
                    nc.vector.tensor_sub(out_t[:, sl], tmp[:], bco2[:])
                else:
                    t2 = dpool.tile([C, FS], dt.float32, tag="lntmp2")
                    nc.vector.tensor_sub(t2[:], tmp[:], bco2[:])
                    nc.vector.tensor_scalar(out_t[:, sl], t2[:], gb[:, 0:1],
                                            gb[:, 1:2], Alu.mult, Alu.add)

        def gnconv(br, xin_bf, dp_bf, h_out):
            """h_out (bf16) = leaky_relu(gnconv_br(xin, dp), 0.01)."""
            # proj_in: two 128-row m-groups; lo rows = [abc 0:120 | pwa 0:8],
            # hi rows = abc 120:248. Bias added during the psum->padded copy.
            for mg, dst in ((0, flo_p), (1, fhi_p)):
                for j in range(NB):
                    pt = ps.tile([C, FB], dt.float32, tag="mm")
                    sl = slice(j * FB, (j + 1) * FB)
                    w0 = t_piw[:, (br * 4 + 0 * 2 + mg) * C:(br * 4 + 0 * 2 + mg + 1) * C]
                    w1 = t_piw[:, (br * 4 + 1 * 2 + mg) * C:(br * 4 + 1 * 2 + mg + 1) * C]
                    nc.tensor.matmul(pt[:], w0, xin_bf[:, sl], start=True, stop=False)
                    nc.tensor.matmul(pt[:], w1, dp_bf[:, sl], start=False, stop=True)
                    rows = FB // W  # 8
                    y0 = j * rows
                    dst_ap = win(dst[:], 70, DW_W, 3 + y0, rows, 3, W)
                    nc.scalar.activation(dst_ap, win(pt[:], rows, W, 0, rows, 0, W),
                                         Act.Identity,
                                         bias=t_pib[:, br * 2 + mg:br * 2 + mg + 1])
            # pwa again at base partition 0 (y1 needs it aligned with acc_lo[0:8])
            pwa_t = dpool.tile([8, PIX], dt.bfloat16, tag="pwa")
            for j in range(NB):
                pt = ps.tile([8, FB], dt.float32, tag="mm")
                sl = slice(j * FB, (j + 1) * FB)
                nc.tensor.matmul(pt[:], t_pwaw[:, (br * 2 + 0) * 8:(br * 2 + 1) * 8],
                                 xin_bf[:, sl], start=True, stop=False)
                nc.tensor.matmul(pt[:], t_pwaw[:, (br * 2 + 1) * 8:(br * 2 + 2) * 8],
                                 dp_bf[:, sl], start=False, stop=True)
                nc.scalar.activation(pwa_t[:, sl], pt[:], Act.Identity,
                                     bias=t_pwab[:, br:br + 1])
            # depthwise 7x7 MACs: full 128 partitions (pwa rows have zero taps)
            acc_lo = dpool.tile([C, PIX], dt.bfloat16, tag="acclo")
            acc_hi = dpool.tile([C, PIX], dt.bfloat16, tag="acchi")
            # tap split: DVE gets 2x-eligible even-dx taps, PE (diag matmuls)
            # gets the rest; PE partials merge into acc via in-place TT-adds.
            DVE_TAPS = {0: list(range(49)), 1: []}
            for grp, (srcp, acc) in ((0, (flo_p, acc_lo)), (1, (fhi_p, acc_hi))):
                av = win(acc[:], H, W, 0, H, 0, W)
                dtap = DVE_TAPS[grp]
                ptap = [k for k in range(49) if k not in dtap]
                for i, k in enumerate(dtap):
                    dy, dx = k // 7, k % 7
                    s = win(srcp[:], 70, DW_W, dy, H, dx, W)
                    wv = t_dww[:, (br * 2 + grp) * 49 + k:(br * 2 + grp) * 49 + k + 1]
                    if i == 0:
                        nc.vector.tensor_scalar(av, s, wv, None, Alu.mult)
                    else:
                        nc.vector.scalar_tensor_tensor(av, s, wv, av, Alu.mult, Alu.add)
                if not ptap:
                    continue
                base = (br * 49 + 0) * C
                dwd_t = dpool.tile([C, len(ptap) * C], dt.bfloat16, tag="dwd",
                                   name=f"dwd{br}_{grp}")
                nc.sync.dma_start(out=dwd_t[:],
                                  in_=dwdiag.ap()[:, base:base + len(ptap) * C])
                for half in range(4):
                    pts = [ps4.tile([C, FB], dt.float32, tag="dwh",
                                    name=f"dwh{br}_{grp}_{half}_{jj}") for jj in range(2)]
                    for i, k in enumerate(ptap):
                        dy, dx = k // 7, k % 7
                        wv = dwd_t[:, i * C:(i + 1) * C]
                        for jj in range(2):
                            j = half * 2 + jj
                            rows = FB // W  # 8
                            s = win(srcp[:], 70, DW_W, dy + j * rows, rows, dx, W)
                            nc.tensor.matmul(pts[jj][:], wv, s,
                                             start=(i == 0), stop=(i == len(ptap) - 1))
                    for jj in range(2):
                        j = half * 2 + jj
                        sl = slice(j * FB, (j + 1) * FB)
                        if dtap:
                            nc.vector.tensor_add(acc[:, sl], acc[:, sl], pts[jj][:])
                        else:
                            nc.scalar.activation(acc[:, sl], pts[jj][:], Act.Copy)
            # y1 = pwa * (dw0 + b0): all operands on partitions 0..7
            y1 = dpool.tile([8, PIX], dt.bfloat16, tag="scratch")
            nc.vector.scalar_tensor_tensor(y1[:], acc_lo[0:8, :],
                                           t_dwb0[0:8, br:br + 1], pwa_t[:],
                                           Alu.add, Alu.mult)
            # composed matmuls (+ bconst, leaky) -> h_out
            rhs_list = [y1[0:8, :], acc_lo[:, :], acc_hi[:, :]]
            for j in range(NB):
                pt = ps.tile([C, FB], dt.float32, tag="mm")
                sl = slice(j * FB, (j + 1) * FB)
                for ti in range(3):
                    kk = COMP_K[ti]
                    wv = t_comp[0:kk, (br * 3 + ti) * C:(br * 3 + ti + 1) * C]
                    nc.tensor.matmul(pt[:], wv, rhs_list[ti][:, sl],
                                     start=(ti == 0), stop=(ti == 2))
                nc.scalar.activation(h_out[:, sl], pt[:], Act.Lrelu,
                                     bias=t_bc[:, br:br + 1], alpha=0.01)

        # ================= per-sample loop =================
        for s in range(SPB):
            xs = dpool.tile([C, PIX], dt.float32, tag="xs")
            nc.sync.dma_start(out=xs[:], in_=x_in.ap()[s])
            rps = dpool.tile([C, PIX], dt.bfloat16, tag="rps")
            nc.sync.dma_start(out=rps[:], in_=rp_in.ap()[s])
            dps = dpool.tile([C, PIX], dt.bfloat16, tag="dps")
            nc.sync.dma_start(out=dps[:], in_=cp_in.ap()[s])

            # ---- decision network ----
            d0 = dn_pads[0]
            din_ap = win(d0[:], 66, 66, 1, H, 1, W)
            nc.vector.tensor_add(din_ap, win(xs[:], H, W, 0, H, 0, W),
                                 win(rps[:], H, W, 0, H, 0, W))
            m2t = None
            for st, (ci, co, side) in enumerate(DN_STAGES):
                n_f = side * side
                fch = min(FB, n_f)
                nchk = n_f // fch
                rows = side // nchk
                pw_ = side + 2
                cur = dn_pads[st]
                scr = dpool.tile([co, n_f], dt.bfloat16, tag="scratch")
                for j in range(nchk):
                    pt = ps.tile([co, fch], dt.float32, tag="mm")
                    for t9 in range(9):
                        dy, dxx = t9 // 3, t9 % 3
                        src = win(cur[0:ci, :], pw_, pw_, dy + j * rows, rows, dxx, side)
                        nc.tensor.matmul(pt[:], w_dn[st][:, t9 * co:(t9 + 1) * co],
                                         src, start=(t9 == 0), stop=(t9 == 8))
                    nc.scalar.activation(scr[:, j * fch:(j + 1) * fch], pt[:],
                                         Act.Relu, bias=b_dn[st])
                hs = side // 2
                m1 = dpool.tile([co, side * hs], dt.bfloat16, tag="dnm1")
                sv = scr[:].rearrange("p (r c2 two) -> p r c2 two", r=side, c2=hs, two=2)
                nc.vector.tensor_max(win(m1[:], side, hs, 0, side, 0, hs),
                                     sv[:, :, :, 0], sv[:, :, :, 1])
                mv = m1[:].rearrange("p (r2 two c) -> p r2 two c", r2=hs, two=2, c=hs)
                if st < 3:
                    nxt = dn_pads[st + 1]
                    pwn = DN_STAGES[st + 1][2] + 2
                    dst = win(nxt[0:co, :], pwn, pwn, 1, hs, 1, hs)
                    nc.vector.tensor_max(dst, mv[:, :, 0, :], mv[:, :, 1, :])
                else:
                    m2t = dpool.tile([8, 16], dt.float32, tag="dnm2")
                    nc.vector.tensor_max(win(m2t[:], 4, 4, 0, 4, 0, 4),
                                         mv[:, :, 0, :], mv[:, :, 1, :])
            hmean = dpool.tile([8, 1], dt.float32, tag="hmean")
            nc.vector.tensor_reduce(hmean[:], m2t[:], mybir.AxisListType.X, Alu.add)
            hmean_bf = dpool.tile([8, 1], dt.bfloat16, tag="hmeanbf")
            nc.vector.tensor_scalar(hmean_bf[:], hmean[:], 1.0 / 16.0, None, Alu.mult)
            ldp = ps.tile([1, 1], dt.float32, tag="mm")
            nc.tensor.matmul(ldp[:], w_d4[:], hmean_bf[:], start=True, stop=True)
            # sdec = (ldiff > thr') * rs
            sdec = dpool.tile([1, 1], dt.float32, tag="sdec")
            nc.vector.tensor_scalar(sdec[:], ldp[:], t_thr[0:1, s:s + 1], t_rs[:],
                                    Alu.is_gt, Alu.mult)
            sdec_b = dpool.tile([C, 1], dt.float32, tag="sdecb")
            nc.gpsimd.partition_broadcast(sdec_b[:], sdec[:])

            # ---- ADB ----
            inputs_t = dpool.tile([C, PIX], dt.bfloat16, tag="inputs")
            ln_norm(xs, True, inputs_t, t_ln1)
            h_t = dpool.tile([C, PIX], dt.bfloat16, tag="h")
            gnconv(0, inputs_t, dps, h_t)
            y_t = dpool.tile([C, PIX], dt.bfloat16, tag="y")
            nc.vector.scalar_tensor_tensor(y_t[:], h_t[:], t_bg[:, 0:1], inputs_t[:],
                                           Alu.mult, Alu.add)
            n2_t = dpool.tile([C, PIX], dt.bfloat16, tag="pwa")
            ln_norm(h_t, False, n2_t, None)
            h2a_t = dpool.tile([C, PIX], dt.bfloat16, tag="h2a")
            for j in range(NB):
                pt = ps.tile([C, FB], dt.float32, tag="mm")
                sl = slice(j * FB, (j + 1) * FB)
                nc.tensor.matmul(pt[:], t_c11[:], n2_t[:, sl], start=True, stop=True)
                nc.scalar.activation(h2a_t[:, sl], pt[:], Act.Identity, bias=t_c11b[:])
            h2_t = dpool.tile([C, PIX], dt.bfloat16, tag="inputs")
            gnconv(1, h2a_t, dps, h2_t)
            a_t = dpool.tile([C, PIX], dt.bfloat16, tag="A")
            nc.vector.scalar_tensor_tensor(a_t[:], h2_t[:], t_bg[:, 1:2], y_t[:],
                                           Alu.mult, Alu.add)
            o_t = dpool.tile([C, PIX], dt.float32, tag="o")
            nc.vector.scalar_tensor_tensor(o_t[:], a_t[:], sdec_b[:], xs[:],
                                           Alu.mult, Alu.add)
            nc.sync.dma_start(out=out_d.ap()[s], in_=o_t[:])

        ctx.close()

    nc.compile()
    return nc


# ------------------------------------------------------------------
def prep_inputs(x, cPromt, rPromt, params):
    import jax
    import ml_dtypes

    bf16 = ml_dtypes.bfloat16
    g = lambda a: np.asarray(a, np.float32)

    P = {}
    for i, (ci, co, _) in enumerate(DN_STAGES):
        w, b = params["dn"][i]
        w = g(w)
        lhs = np.zeros((ci, 9 * co), np.float32)
        for t in range(9):
            lhs[:, t * co:(t + 1) * co] = w[:, :, t // 3, t % 3].T
        P[f"dn_w{i}"] = lhs.astype(bf16)
        P[f"dn_b{i}"] = g(b).reshape(-1, 1)
    w4, b4 = params["dn"][4]
    w4 = g(w4)[:, :, 0, 0]
    P["dn4_wd"] = (w4[1] - w4[0]).reshape(8, 1).astype(bf16)
    bd = float(g(b4)[1] - g(b4)[0])

    # The reference's gumbel noise: jax threefry streams differ between the
    # axon-neuron backend and CPU. The harness's reference runs on CPU (the
    # model cannot jit-compile on neuron), so pin the CPU stream explicitly.
    with jax.default_device(jax.devices("cpu")[0]):
        u = np.asarray(jax.random.uniform(jax.random.key(7), (B, 2), np.float32,
                                          1e-6, 1.0 - 1e-6), np.float64)
    gum = -np.log(-np.log(u))
    G = gum[:, 0] - gum[:, 1]
    thr = np.where(np.abs(G) < 1.0,
                   2.0 * np.arctanh(np.clip(G, -1 + 1e-12, 1 - 1e-12)),
                   np.where(G >= 1.0, 1e30, -1e30))
    P["rs"] = np.full((1, 1), float(np.asarray(params["res_scale"])), np.float32)

    adb = params["adb"]
    P["ln1_gb"] = np.stack([g(adb["ln1"][0]), g(adb["ln1"][1])], 1).astype(np.float32)
    P["eps"] = np.full((8, 1), 1e-6, np.float32)
    P["ones_bf"] = np.full((C, 8), 1.0 / C, np.float32).astype(bf16)
    oh = np.zeros((8, C), np.float32)
    oh[0, :] = 1.0
    P["onehot"] = oh.astype(bf16)
    ln2_g, ln2_b = g(adb["ln2"][0]), g(adb["ln2"][1])
    W11 = g(adb["conv11"][0])[:, :, 0, 0]
    P["c11_wT"] = (W11 * ln2_g[None, :]).T.astype(bf16)
    P["c11_b"] = (g(adb["conv11"][1]) + W11 @ ln2_b).reshape(-1, 1).astype(np.float32)
    P["beta_g"] = np.stack([g(adb["beta"]).reshape(C), g(adb["gamma"]).reshape(C)],
                           1).astype(np.float32)

    pi_w = np.zeros((C, 8 * C), np.float32)
    pi_b = np.zeros((C, 4), np.float32)
    dww = np.zeros((C, 4 * 49), np.float32)
    dwb0 = np.zeros((C, 2), np.float32)
    comp_w = np.zeros((C, 6 * C), np.float32)
    dwdiag_h = np.zeros((C, 98 * C), np.float32)
    pwa_w = np.zeros((C, 2 * 2 * 8), np.float32)
    pwab = np.zeros((8, 2), np.float32)
    bconst = np.zeros((C, 2), np.float32)
    # lo m-group row r: abc channel r (fused 8+r) for r<120, pwa r-120 for r>=120
    perm_lo = np.array([8 + r for r in range(120)] + [r for r in range(8)])
    for bi, name in enumerate(("b1", "b2")):
        q = adb[name]
        Wpi = g(q["proj_in_mf"][0])[:, :, 0, 0]
        bpi = g(q["proj_in_mf"][1])
        for kg in range(2):
            blk = Wpi[:, kg * C:(kg + 1) * C]
            for mg in range(2):
                col = (bi * 4 + kg * 2 + mg) * C
                rows = perm_lo if mg == 0 else np.arange(C, 2 * C)
                pi_w[:, col:col + C] = blk[rows, :].T
            pwa_w[:, (bi * 2 + kg) * 8:(bi * 2 + kg + 1) * 8] = blk[0:8, :].T
        pi_b[:, bi * 2 + 0] = bpi[perm_lo]
        pi_b[:, bi * 2 + 1] = bpi[C:2 * C]
        pwab[:, bi] = bpi[0:8]
        wdw = g(q["dwconv"][0])[:, 0].reshape(248, 49)
        dww[0:120, (bi * 2 + 0) * 49:(bi * 2 + 1) * 49] = wdw[0:120]
        dww[:, (bi * 2 + 1) * 49:(bi * 2 + 2) * 49] = wdw[120:248]
        for i in range(49):
            col = (bi * 49 + i) * C
            np.fill_diagonal(dwdiag_h[:, col:col + C], wdw[120:248, i])
        bdw = g(q["dwconv"][1])
        dwb0[0:8, bi] = bdw[0:8]
        pw = [g(q["pws"][i][0])[:, :, 0, 0] for i in range(4)]
        pwb = [g(q["pws"][i][1]) for i in range(4)]
        Wpo = g(q["proj_out"][0])[:, :, 0, 0]
        bpo = g(q["proj_out"][1])
        C4 = Wpo @ pw[3]; C3 = C4 @ pw[2]; C2 = C3 @ pw[1]; C1 = C2 @ pw[0]
        bc = (Wpo @ pwb[3] + C4 @ pwb[2] + C3 @ pwb[1] + C2 @ pwb[0] + bpo
              + C2 @ bdw[8:24] + C3 @ bdw[24:56] + C4 @ bdw[56:120]
              + Wpo @ bdw[120:248])
        # term 0: y1 (K=8); term 1: merged lo (K=128, rows=acc_lo channels);
        # term 2: hi (K=128)
        comp_w[0:8, (bi * 3 + 0) * C:(bi * 3 + 1) * C] = C1.T
        merged = np.zeros((C, C), np.float32)
        merged[8:24, :] = C2.T       # dw1 = abc 8:24 at partitions 8:24
        merged[24:56, :] = C3.T
        merged[56:120, :] = C4.T
        comp_w[:, (bi * 3 + 1) * C:(bi * 3 + 2) * C] = merged
        comp_w[:, (bi * 3 + 2) * C:(bi * 3 + 3) * C] = Wpo.T
        bconst[:, bi] = bc
    P["pi_w"] = pi_w.astype(bf16)
    P["pi_w"] = pi_w.astype(bf16)
    P["pi_b"] = pi_b
    P["dww"] = dww
    P["dwb0"] = dwb0
    P["comp_w"] = comp_w.astype(bf16)
    P["dwdiag"] = dwdiag_h.astype(bf16)
    P["pwa_w"] = pwa_w.astype(bf16)
    P["pwab"] = pwab
    P["bconst"] = bconst

    x = np.asarray(x, np.float32).reshape(B, C, PIX)
    cp = np.asarray(cPromt, np.float32).reshape(B, C, PIX).astype(bf16)
    rp = np.asarray(rPromt, np.float32).reshape(B, C, PIX).astype(bf16)

    per_core = []
    for c in range(N_CORES):
        idx = [c + N_CORES * k for k in range(SPB)]
        m = dict(P)
        m["x_in"] = np.ascontiguousarray(x[idx])
        m["cp_in"] = np.ascontiguousarray(cp[idx])
        m["rp_in"] = np.ascontiguousarray(rp[idx])
        m["thr"] = (thr[idx] - bd).reshape(1, SPB).astype(np.float32)
        per_core.append(m)
    return per_core


_NC_CACHE = {}


def kernel(x, cPromt, rPromt, params):
    from concourse.bass_utils import run_bass_kernel_spmd

    per_core = prep_inputs(x, cPromt, rPromt, params)
    if "nc" not in _NC_CACHE:
        _NC_CACHE["nc"] = build_nc()
    nc = _NC_CACHE["nc"]
    res = run_bass_kernel_spmd(nc, per_core, list(range(N_CORES)))
    out = np.zeros((B, C, PIX), np.float32)
    for c in range(N_CORES):
        oc = res.results[c]["out"]
        for k in range(SPB):
            out[c + N_CORES * k] = oc[k]
    return out.reshape(B, C, H, W)


# revision 24
# speedup vs baseline: 2144.2837x; 2144.2837x over previous
"""Trainium2 Bass kernel for nn_D3Net (moe_routing).

Data-parallel over 8 NeuronCores: core c takes samples [c, c+8, c+16, c+24].
Per sample: decision network -> gate; ADB branch (2x gnconv with depthwise
7x7); gated residual combine.

Layout: channels on SBUF partitions, pixels (y-major) on the free dim.
Dense convs/matmuls in bf16 (PE, 1 cyc/row); LN statistics in fp32r.
Depthwise 7x7 = 49 scalar_tensor_tensor MACs per channel group:
low group (abc 0..119, partitions 8..127) on VectorE, high group
(abc 120..247) on GpSimd, in bf16.

`kernel(**inputs)` is the host entry: preps weight layouts, compiles once,
runs SPMD on cores 0-7 via run_bass_kernel_spmd, reassembles full output.
"""

import sys

sys.path.insert(0, "/opt/trn_rl_repo")

import numpy as np

N_CORES = 8
B, C, H, W = 32, 128, 64, 64
PIX = H * W            # 4096
SPB = B // N_CORES     # 4 samples per core

# padded abc layout for depthwise 7x7: 70 rows x 72 cols, interior at (3, 3)
DW_W = 72
DW_F = 70 * DW_W       # 5040
DW_BASE = 3 * DW_W + 3

# dn stages: (in_ch, out_ch, in_side); padded input tile: (side+2) x (side+2)
DN_STAGES = [(128, 64, 64), (64, 32, 32), (32, 16, 16), (16, 8, 8)]

FB = 512    # f-chunk for matmul moving operands (ISA limit)
NB = PIX // FB  # 8
FS = 512    # f-chunk for fp32r LN statistic matmuls
NS = PIX // FS  # 8
COMP_K = [8, 128, 128]


def build_nc():
    from contextlib import ExitStack

    import concourse.bacc as bacc
    import concourse.mybir as mybir
    from concourse import tile

    dt = mybir.dt
    Alu = mybir.AluOpType
    Act = mybir.ActivationFunctionType
    f32r = dt.float32r

    nc = bacc.Bacc("TRN2", target_bir_lowering=False, debug=False,
                   num_devices=N_CORES)

    def din(name, shape, d=dt.float32):
        return nc.dram_tensor(name, shape, d, kind="ExternalInput")

    x_in = din("x_in", (SPB, C, PIX))
    rp_in = din("rp_in", (SPB, C, PIX), dt.bfloat16)
    cp_in = din("cp_in", (SPB, C, PIX), dt.bfloat16)
    out_d = nc.dram_tensor("out", (SPB, C, PIX), dt.float32, kind="ExternalOutput")

    dn_w = [din(f"dn_w{i}", (ci, 9 * co), dt.bfloat16) for i, (ci, co, _) in enumerate(DN_STAGES)]
    dn_b = [din(f"dn_b{i}", (co, 1)) for i, (_, co, _) in enumerate(DN_STAGES)]
    dn4_wd = din("dn4_wd", (8, 1), dt.bfloat16)
    thr_in = din("thr", (1, SPB))        # thr - bd, pre-adjusted on host
    rs_in = din("rs", (1, 1))            # res_scale
    ln1_gb = din("ln1_gb", (C, 2))
    eps_in = din("eps", (8, 1))
    ones_bf = din("ones_bf", (C, 8), dt.bfloat16)
    onehot = din("onehot", (8, C), dt.bfloat16)  # bcast lhsT: row-0 selector
    pi_w = din("pi_w", (C, 8 * C), dt.bfloat16)   # [(ci%128), (br, kg, mg, co)]
    pi_b = din("pi_b", (C, 4))                    # [(co%128), (br, mg)]
    dww = din("dww", (C, 4 * 49))                 # [(part), (br, grp, tap)]
    dwb0 = din("dwb0", (C, 2))                    # rows 8:16 = dw bias 0:8
    comp_w = din("comp_w", (C, 6 * C), dt.bfloat16)   # [(k), (br, term{y1,lo,hi}, co)]
    pwa_w = din("pwa_w", (C, 2 * 2 * 8), dt.bfloat16)  # [(ci), (br, kg, 8)]
    pwab = din("pwab", (8, 2))                         # pwa bias [8, br]
    bconst = din("bconst", (C, 2))
    dwdiag = din("dwdiag", (C, 98 * C), dt.bfloat16)  # PE-tap diags, packed
    beta_g = din("beta_g", (C, 2))
    c11_wT = din("c11_wT", (C, C), dt.bfloat16)
    c11_b = din("c11_b", (C, 1))

    ctx = ExitStack()
    with tile.TileContext(nc, pool_alloc_mode="queue") as tc:
        wp = ctx.enter_context(tc.tile_pool(name="wpool", bufs=1))
        pp = ctx.enter_context(tc.tile_pool(name="perm", bufs=1))
        dpool = ctx.enter_context(tc.tile_pool(name="data", bufs=1))
        db2 = ctx.enter_context(tc.tile_pool(name="dbuf", bufs=2))
        ps = ctx.enter_context(tc.tile_pool(name="psum", bufs=2, space="PSUM"))
        ps4 = ctx.enter_context(tc.tile_pool(name="psum4", bufs=2, space="PSUM"))
        ps1 = ctx.enter_context(tc.tile_pool(name="psum1", bufs=1, space="PSUM"))

        def wtile(src, shape):
            t = wp.tile(list(shape),
                        src.tensor.dtype if hasattr(src, "tensor") else src.dtype,
                        tag=f"w_{src.name}")
            nc.sync.dma_start(out=t[:], in_=src.ap())
            return t

        w_dn = [wtile(dn_w[i], (DN_STAGES[i][0], 9 * DN_STAGES[i][1])) for i in range(4)]
        b_dn = [wtile(dn_b[i], (DN_STAGES[i][1], 1)) for i in range(4)]
        w_d4 = wtile(dn4_wd, (8, 1))
        t_thr = wtile(thr_in, (1, SPB))
        t_rs = wtile(rs_in, (1, 1))
        t_ln1 = wtile(ln1_gb, (C, 2))
        t_eps = wtile(eps_in, (8, 1))
        t_onb = wtile(ones_bf, (C, 8))
        t_oh = wtile(onehot, (8, C))
        t_piw = wtile(pi_w, (C, 8 * C))
        t_pib = wtile(pi_b, (C, 4))
        t_dww = wtile(dww, (C, 4 * 49))
        t_dwb0 = wtile(dwb0, (C, 2))
        t_comp = wtile(comp_w, (C, 6 * C))
        t_pwaw = wtile(pwa_w, (C, 2 * 2 * 8))
        t_pwab = wtile(pwab, (8, 2))
        t_bc = wtile(bconst, (C, 2))
        t_bg = wtile(beta_g, (C, 2))
        t_c11 = wtile(c11_wT, (C, C))
        t_c11b = wtile(c11_b, (C, 1))

        # persistent padded tiles; zero once, interiors rewritten per sample
        dn_pads = []
        for si, (ci, _, side) in enumerate(DN_STAGES):
            t = pp.tile([ci, (side + 2) * (side + 2)], dt.bfloat16, tag=f"dnpad{si}")
            nc.vector.memset(t[:], 0.0)
            dn_pads.append(t)
        flo_p = pp.tile([C, DW_F], dt.bfloat16, tag="flo")
        fhi_p = pp.tile([C, DW_F], dt.bfloat16, tag="fhi")
        flo_o = pp.tile([C, DW_F], dt.bfloat16, tag="floo")
        nc.vector.memset(flo_p[:], 0.0)
        nc.vector.memset(fhi_p[:], 0.0)
        nc.vector.memset(flo_o[:], 0.0)

        def win(ap_2d, rows, stride, r0, nr, c0, ncol):
            """3D window view of a flat [P, rows*stride] AP."""
            v = ap_2d.rearrange("p (r c) -> p r c", r=rows, c=stride)
            return v[:, r0:r0 + nr, c0:c0 + ncol]

        def ln_norm(src, src_is_f32, out_t, gb):
            """out = (src - mu(ch)) * rstd(ch) [* g + b]; src [C, PIX] tile.

            Per 512-pixel chunk: matmul with [C, 8] ones/C -> psum [8, FS]
            (8 identical rows, base partition 0), compact rsqrt via
            exp(-0.5*ln(var+eps)), broadcast back to 128 partitions via a
            one-hot-row matmul, then normalize with two DVE passes.
            """
            xsq = dpool.tile([C, PIX], dt.bfloat16, tag="scratch")
            nc.scalar.square(xsq[:], src[:])
            if src_is_f32:
                src_bf = dpool.tile([C, PIX], dt.bfloat16, tag="srcbf")
                nc.vector.tensor_copy(src_bf[:], src[:])
            else:
                src_bf = src
            for j in range(NS):
                sl = slice(j * FS, (j + 1) * FS)
                stats = ps1.tile([8, 2 * FS], dt.float32, tag="auxs")
                nc.tensor.matmul(stats[:, 0:FS], t_onb[:],
                                 src_bf[:, sl], start=True, stop=True)
                nc.tensor.matmul(stats[:, FS:2 * FS], t_onb[:],
                                 xsq[:, sl], start=True, stop=True)
                musq = dpool.tile([8, FS], dt.float32, tag="musq")
                nc.scalar.square(musq[:], stats[:, 0:FS])
                var_c = dpool.tile([8, FS], dt.float32, tag="varc")
                nc.vector.scalar_tensor_tensor(var_c[:], musq[:], -1.0,
                                               stats[:, FS:2 * FS], Alu.mult, Alu.add)
                lnv = dpool.tile([8, FS], dt.float32, tag="lnv")
                nc.scalar.activation(lnv[:], var_c[:], Act.Ln, bias=t_eps[:])
                rstd_c = dpool.tile([8, FS], dt.bfloat16, tag="rstdc")
                nc.scalar.activation(rstd_c[:], lnv[:], Act.Exp, scale=-0.5)
                murstd_c = dpool.tile([8, FS], dt.bfloat16, tag="murstdc")
                nc.vector.tensor_mul(murstd_c[:], stats[:, 0:FS], rstd_c[:])
                bco = ps1.tile([C, FS], dt.float32, tag="auxb")
                nc.tensor.matmul(bco[:], t_oh[:], rstd_c[:], start=True, stop=True)
                tmp = dpool.tile([C, FS], dt.float32, tag="lntmp")
                nc.vector.tensor_mul(tmp[:], src[:, sl], bco[:])
                bco2 = ps1.tile([C, FS], dt.float32, tag="auxb")
                nc.tensor.matmul(bco2[:], t_oh[:], murstd_c[:], start=True, stop=True)
                if gb is None:
                    nc.vector.tensor_sub(out_t[:, sl], tmp[:], bco2[:])
                else:
                    t2 = dpool.tile([C, FS], dt.float32, tag="lntmp2")
                    nc.vector.tensor_sub(t2[:], tmp[:], bco2[:])
                    nc.vector.tensor_scalar(out_t[:, sl], t2[:], gb[:, 0:1],
                                            gb[:, 1:2], Alu.mult, Alu.add)

        def gnconv(br, xin_bf, dp_bf, h_out):
            """h_out (bf16) = leaky_relu(gnconv_br(xin, dp), 0.01)."""
            # proj_in: two 128-row m-groups; lo rows = [abc 0:120 | pwa 0:8],
            # hi rows = abc 120:248. Bias added during the psum->padded copy.
            for mg, dst in ((0, flo_p), (1, fhi_p)):
                for j in range(NB):
                    pt = ps.tile([C, FB], dt.float32, tag="mm")
                    sl = slice(j * FB, (j + 1) * FB)
                    w0 = t_piw[:, (br * 4 + 0 * 2 + mg) * C:(br * 4 + 0 * 2 + mg + 1) * C]
                    w1 = t_piw[:, (br * 4 + 1 * 2 + mg) * C:(br * 4 + 1 * 2 + mg + 1) * C]
                    nc.tensor.matmul(pt[:], w0, xin_bf[:, sl], start=True, stop=False)
                    nc.tensor.matmul(pt[:], w1, dp_bf[:, sl], start=False, stop=True)
                    rows = FB // W  # 8
                    y0 = j * rows
                    dst_ap = win(dst[:], 70, DW_W, 3 + y0, rows, 3, W)
                    nc.scalar.activation(dst_ap, win(pt[:], rows, W, 0, rows, 0, W),
                                         Act.Identity,
                                         bias=t_pib[:, br * 2 + mg:br * 2 + mg + 1])
            # shifted-by-one copy of flo so odd-dx taps read 4B-aligned (2x mode)
            nc.scalar.activation(flo_o[:, 0:DW_F - 1], flo_p[:, 1:DW_F], Act.Copy)
            # pwa again at base partition 0 (y1 needs it aligned with acc_lo[0:8])
            pwa_t = dpool.tile([8, PIX], dt.bfloat16, tag="pwa")
            for j in range(NB):
                pt = ps.tile([8, FB], dt.float32, tag="mm")
                sl = slice(j * FB, (j + 1) * FB)
                nc.tensor.matmul(pt[:], t_pwaw[:, (br * 2 + 0) * 8:(br * 2 + 1) * 8],
                                 xin_bf[:, sl], start=True, stop=False)
                nc.tensor.matmul(pt[:], t_pwaw[:, (br * 2 + 1) * 8:(br * 2 + 2) * 8],
                                 dp_bf[:, sl], start=False, stop=True)
                nc.scalar.activation(pwa_t[:, sl], pt[:], Act.Identity,
                                     bias=t_pwab[:, br:br + 1])
            # depthwise 7x7 MACs: full 128 partitions (pwa rows have zero taps)
            acc_lo = dpool.tile([C, PIX], dt.bfloat16, tag="acclo")
            acc_hi = dpool.tile([C, PIX], dt.bfloat16, tag="acchi")
            # tap split: DVE gets 2x-eligible even-dx taps, PE (diag matmuls)
            # gets the rest; PE partials merge into acc via in-place TT-adds.
            DVE_TAPS = {0: list(range(49)), 1: []}
            for grp, (srcp, acc) in ((0, (flo_p, acc_lo)), (1, (fhi_p, acc_hi))):
                av = win(acc[:], H, W, 0, H, 0, W)
                dtap = DVE_TAPS[grp]
                ptap = [k for k in range(49) if k not in dtap]
                for i, k in enumerate(dtap):
                    dy, dx = k // 7, k % 7
                    if grp == 0 and dx % 2 == 1:
                        s = win(flo_o[:], 70, DW_W, dy, H, dx - 1, W)
                    else:
                        s = win(srcp[:], 70, DW_W, dy, H, dx, W)
                    wv = t_dww[:, (br * 2 + grp) * 49 + k:(br * 2 + grp) * 49 + k + 1]
                    if i == 0:
                        nc.vector.tensor_scalar(av, s, wv, None, Alu.mult)
                    else:
                        nc.vector.scalar_tensor_tensor(av, s, wv, av, Alu.mult, Alu.add)
                if not ptap:
                    continue
                base = (br * 49 + 0) * C
                dwd_t = dpool.tile([C, len(ptap) * C], dt.bfloat16, tag="dwd",
                                   name=f"dwd{br}_{grp}")
                nc.sync.dma_start(out=dwd_t[:],
                                  in_=dwdiag.ap()[:, base:base + len(ptap) * C])
                for half in range(4):
                    pts = [ps4.tile([C, FB], dt.float32, tag="dwh",
                                    name=f"dwh{br}_{grp}_{half}_{jj}") for jj in range(2)]
                    for i, k in enumerate(ptap):
                        dy, dx = k // 7, k % 7
                        wv = dwd_t[:, i * C:(i + 1) * C]
                        for jj in range(2):
                            j = half * 2 + jj
                            rows = FB // W  # 8
                            s = win(srcp[:], 70, DW_W, dy + j * rows, rows, dx, W)
                            nc.tensor.matmul(pts[jj][:], wv, s,
                                             start=(i == 0), stop=(i == len(ptap) - 1))
                    for jj in range(2):
                        j = half * 2 + jj
                        sl = slice(j * FB, (j + 1) * FB)
                        if dtap:
                            nc.vector.tensor_add(acc[:, sl], acc[:, sl], pts[jj][:])
                        else:
                            nc.scalar.activation(acc[:, sl], pts[jj][:], Act.Copy)
            # y1 = pwa * (dw0 + b0): all operands on partitions 0..7
            y1 = dpool.tile([8, PIX], dt.bfloat16, tag="scratch")
            nc.vector.scalar_tensor_tensor(y1[:], acc_lo[0:8, :],
                                           t_dwb0[0:8, br:br + 1], pwa_t[:],
                                           Alu.add, Alu.mult)
            # composed matmuls (+ bconst, leaky) -> h_out
            rhs_list = [y1[0:8, :], acc_lo[:, :], acc_hi[:, :]]
            for j in range(NB):
                pt = ps.tile([C, FB], dt.float32, tag="mm")
                sl = slice(j * FB, (j + 1) * FB)
                for ti in range(3):
                    kk = COMP_K[ti]
                    wv = t_comp[0:kk, (br * 3 + ti) * C:(br * 3 + ti + 1) * C]
                    nc.tensor.matmul(pt[:], wv, rhs_list[ti][:, sl],
                                     start=(ti == 0), stop=(ti == 2))
                nc.scalar.activation(h_out[:, sl], pt[:], Act.Lrelu,
                                     bias=t_bc[:, br:br + 1], alpha=0.01)

        # ================= per-sample loop =================
        for s in range(SPB):
            xs = dpool.tile([C, PIX], dt.float32, tag="xs")
            nc.sync.dma_start(out=xs[:], in_=x_in.ap()[s])
            rps = dpool.tile([C, PIX], dt.bfloat16, tag="rps")
            nc.sync.dma_start(out=rps[:], in_=rp_in.ap()[s])
            dps = dpool.tile([C, PIX], dt.bfloat16, tag="dps")
            nc.sync.dma_start(out=dps[:], in_=cp_in.ap()[s])

            # ---- decision network ----
            d0 = dn_pads[0]
            din_ap = win(d0[:], 66, 66, 1, H, 1, W)
            nc.vector.tensor_add(din_ap, win(xs[:], H, W, 0, H, 0, W),
                                 win(rps[:], H, W, 0, H, 0, W))
            m2t = None
            for st, (ci, co, side) in enumerate(DN_STAGES):
                n_f = side * side
                fch = min(FB, n_f)
                nchk = n_f // fch
                rows = side // nchk
                pw_ = side + 2
                cur = dn_pads[st]
                scr = dpool.tile([co, n_f], dt.bfloat16, tag="scratch")
                for j in range(nchk):
                    pt = ps.tile([co, fch], dt.float32, tag="mm")
                    for t9 in range(9):
                        dy, dxx = t9 // 3, t9 % 3
                        src = win(cur[0:ci, :], pw_, pw_, dy + j * rows, rows, dxx, side)
                        nc.tensor.matmul(pt[:], w_dn[st][:, t9 * co:(t9 + 1) * co],
                                         src, start=(t9 == 0), stop=(t9 == 8))
                    nc.scalar.activation(scr[:, j * fch:(j + 1) * fch], pt[:],
                                         Act.Relu, bias=b_dn[st])
                hs = side // 2
                m1 = dpool.tile([co, side * hs], dt.bfloat16, tag="dnm1")
                sv = scr[:].rearrange("p (r c2 two) -> p r c2 two", r=side, c2=hs, two=2)
                nc.vector.tensor_max(win(m1[:], side, hs, 0, side, 0, hs),
                                     sv[:, :, :, 0], sv[:, :, :, 1])
                mv = m1[:].rearrange("p (r2 two c) -> p r2 two c", r2=hs, two=2, c=hs)
                if st < 3:
                    nxt = dn_pads[st + 1]
                    pwn = DN_STAGES[st + 1][2] + 2
                    dst = win(nxt[0:co, :], pwn, pwn, 1, hs, 1, hs)
                    nc.vector.tensor_max(dst, mv[:, :, 0, :], mv[:, :, 1, :])
                else:
                    m2t = dpool.tile([8, 16], dt.float32, tag="dnm2")
                    nc.vector.tensor_max(win(m2t[:], 4, 4, 0, 4, 0, 4),
                                         mv[:, :, 0, :], mv[:, :, 1, :])
            hmean = dpool.tile([8, 1], dt.float32, tag="hmean")
            nc.vector.tensor_reduce(hmean[:], m2t[:], mybir.AxisListType.X, Alu.add)
            hmean_bf = dpool.tile([8, 1], dt.bfloat16, tag="hmeanbf")
            nc.vector.tensor_scalar(hmean_bf[:], hmean[:], 1.0 / 16.0, None, Alu.mult)
            ldp = ps.tile([1, 1], dt.float32, tag="mm")
            nc.tensor.matmul(ldp[:], w_d4[:], hmean_bf[:], start=True, stop=True)
            # sdec = (ldiff > thr') * rs
            sdec = dpool.tile([1, 1], dt.float32, tag="sdec")
            nc.vector.tensor_scalar(sdec[:], ldp[:], t_thr[0:1, s:s + 1], t_rs[:],
                                    Alu.is_gt, Alu.mult)
            sdec_b = dpool.tile([C, 1], dt.float32, tag="sdecb")
            nc.gpsimd.partition_broadcast(sdec_b[:], sdec[:])

            # ---- ADB ----
            inputs_t = dpool.tile([C, PIX], dt.bfloat16, tag="inputs")
            ln_norm(xs, True, inputs_t, t_ln1)
            h_t = dpool.tile([C, PIX], dt.bfloat16, tag="h")
            gnconv(0, inputs_t, dps, h_t)
            y_t = dpool.tile([C, PIX], dt.bfloat16, tag="y")
            nc.vector.scalar_tensor_tensor(y_t[:], h_t[:], t_bg[:, 0:1], inputs_t[:],
                                           Alu.mult, Alu.add)
            n2_t = dpool.tile([C, PIX], dt.bfloat16, tag="pwa")
            ln_norm(h_t, False, n2_t, None)
            h2a_t = dpool.tile([C, PIX], dt.bfloat16, tag="h2a")
            for j in range(NB):
                pt = ps.tile([C, FB], dt.float32, tag="mm")
                sl = slice(j * FB, (j + 1) * FB)
                nc.tensor.matmul(pt[:], t_c11[:], n2_t[:, sl], start=True, stop=True)
                nc.scalar.activation(h2a_t[:, sl], pt[:], Act.Identity, bias=t_c11b[:])
            h2_t = dpool.tile([C, PIX], dt.bfloat16, tag="inputs")
            gnconv(1, h2a_t, dps, h2_t)
            a_t = dpool.tile([C, PIX], dt.bfloat16, tag="A")
            nc.vector.scalar_tensor_tensor(a_t[:], h2_t[:], t_bg[:, 1:2], y_t[:],
                                           Alu.mult, Alu.add)
            o_t = dpool.tile([C, PIX], dt.float32, tag="o")
            nc.sync.dma_start(out=o_t[:], in_=x_in.ap()[s])
            nc.vector.scalar_tensor_tensor(o_t[:], a_t[:], sdec_b[:], o_t[:],
                                           Alu.mult, Alu.add)
            nc.sync.dma_start(out=out_d.ap()[s], in_=o_t[:])

        ctx.close()

    nc.compile()
    return nc


# ------------------------------------------------------------------
def prep_inputs(x, cPromt, rPromt, params):
    import jax
    import ml_dtypes

    bf16 = ml_dtypes.bfloat16
    g = lambda a: np.asarray(a, np.float32)

    P = {}
    for i, (ci, co, _) in enumerate(DN_STAGES):
        w, b = params["dn"][i]
        w = g(w)
        lhs = np.zeros((ci, 9 * co), np.float32)
        for t in range(9):
            lhs[:, t * co:(t + 1) * co] = w[:, :, t // 3, t % 3].T
        P[f"dn_w{i}"] = lhs.astype(bf16)
        P[f"dn_b{i}"] = g(b).reshape(-1, 1)
    w4, b4 = params["dn"][4]
    w4 = g(w4)[:, :, 0, 0]
    P["dn4_wd"] = (w4[1] - w4[0]).reshape(8, 1).astype(bf16)
    bd = float(g(b4)[1] - g(b4)[0])

    # The reference's gumbel noise: jax threefry streams differ between the
    # axon-neuron backend and CPU. The harness's reference runs on CPU (the
    # model cannot jit-compile on neuron), so pin the CPU stream explicitly.
    with jax.default_device(jax.devices("cpu")[0]):
        u = np.asarray(jax.random.uniform(jax.random.key(7), (B, 2), np.float32,
                                          1e-6, 1.0 - 1e-6), np.float64)
    gum = -np.log(-np.log(u))
    G = gum[:, 0] - gum[:, 1]
    thr = np.where(np.abs(G) < 1.0,
                   2.0 * np.arctanh(np.clip(G, -1 + 1e-12, 1 - 1e-12)),
                   np.where(G >= 1.0, 1e30, -1e30))
    P["rs"] = np.full((1, 1), float(np.asarray(params["res_scale"])), np.float32)

    adb = params["adb"]
    P["ln1_gb"] = np.stack([g(adb["ln1"][0]), g(adb["ln1"][1])], 1).astype(np.float32)
    P["eps"] = np.full((8, 1), 1e-6, np.float32)
    P["ones_bf"] = np.full((C, 8), 1.0 / C, np.float32).astype(bf16)
    oh = np.zeros((8, C), np.float32)
    oh[0, :] = 1.0
    P["onehot"] = oh.astype(bf16)
    ln2_g, ln2_b = g(adb["ln2"][0]), g(adb["ln2"][1])
    W11 = g(adb["conv11"][0])[:, :, 0, 0]
    P["c11_wT"] = (W11 * ln2_g[None, :]).T.astype(bf16)
    P["c11_b"] = (g(adb["conv11"][1]) + W11 @ ln2_b).reshape(-1, 1).astype(np.float32)
    P["beta_g"] = np.stack([g(adb["beta"]).reshape(C), g(adb["gamma"]).reshape(C)],
                           1).astype(np.float32)

    pi_w = np.zeros((C, 8 * C), np.float32)
    pi_b = np.zeros((C, 4), np.float32)
    dww = np.zeros((C, 4 * 49), np.float32)
    dwb0 = np.zeros((C, 2), np.float32)
    comp_w = np.zeros((C, 6 * C), np.float32)
    dwdiag_h = np.zeros((C, 98 * C), np.float32)
    pwa_w = np.zeros((C, 2 * 2 * 8), np.float32)
    pwab = np.zeros((8, 2), np.float32)
    bconst = np.zeros((C, 2), np.float32)
    # lo m-group row r: abc channel r (fused 8+r) for r<120, pwa r-120 for r>=120
    perm_lo = np.array([8 + r for r in range(120)] + [r for r in range(8)])
    for bi, name in enumerate(("b1", "b2")):
        q = adb[name]
        Wpi = g(q["proj_in_mf"][0])[:, :, 0, 0]
        bpi = g(q["proj_in_mf"][1])
        for kg in range(2):
            blk = Wpi[:, kg * C:(kg + 1) * C]
            for mg in range(2):
                col = (bi * 4 + kg * 2 + mg) * C
                rows = perm_lo if mg == 0 else np.arange(C, 2 * C)
                pi_w[:, col:col + C] = blk[rows, :].T
            pwa_w[:, (bi * 2 + kg) * 8:(bi * 2 + kg + 1) * 8] = blk[0:8, :].T
        pi_b[:, bi * 2 + 0] = bpi[perm_lo]
        pi_b[:, bi * 2 + 1] = bpi[C:2 * C]
        pwab[:, bi] = bpi[0:8]
        wdw = g(q["dwconv"][0])[:, 0].reshape(248, 49)
        dww[0:120, (bi * 2 + 0) * 49:(bi * 2 + 1) * 49] = wdw[0:120]
        dww[:, (bi * 2 + 1) * 49:(bi * 2 + 2) * 49] = wdw[120:248]
        for i in range(49):
            col = (bi * 49 + i) * C
            np.fill_diagonal(dwdiag_h[:, col:col + C], wdw[120:248, i])
        bdw = g(q["dwconv"][1])
        dwb0[0:8, bi] = bdw[0:8]
        pw = [g(q["pws"][i][0])[:, :, 0, 0] for i in range(4)]
        pwb = [g(q["pws"][i][1]) for i in range(4)]
        Wpo = g(q["proj_out"][0])[:, :, 0, 0]
        bpo = g(q["proj_out"][1])
        C4 = Wpo @ pw[3]; C3 = C4 @ pw[2]; C2 = C3 @ pw[1]; C1 = C2 @ pw[0]
        bc = (Wpo @ pwb[3] + C4 @ pwb[2] + C3 @ pwb[1] + C2 @ pwb[0] + bpo
              + C2 @ bdw[8:24] + C3 @ bdw[24:56] + C4 @ bdw[56:120]
              + Wpo @ bdw[120:248])
        # term 0: y1 (K=8); term 1: merged lo (K=128, rows=acc_lo channels);
        # term 2: hi (K=128)
        comp_w[0:8, (bi * 3 + 0) * C:(bi * 3 + 1) * C] = C1.T
        merged = np.zeros((C, C), np.float32)
        merged[8:24, :] = C2.T       # dw1 = abc 8:24 at partitions 8:24
        merged[24:56, :] = C3.T
        merged[56:120, :] = C4.T
        comp_w[:, (bi * 3 + 1) * C:(bi * 3 + 2) * C] = merged
        comp_w[:, (bi * 3 + 2) * C:(bi * 3 + 3) * C] = Wpo.T
        bconst[:, bi] = bc
    P["pi_w"] = pi_w.astype(bf16)
    P["pi_w"] = pi_w.astype(bf16)
    P["pi_b"] = pi_b
    P["dww"] = dww
    P["dwb0"] = dwb0
    P["comp_w"] = comp_w.astype(bf16)
    P["dwdiag"] = dwdiag_h.astype(bf16)
    P["pwa_w"] = pwa_w.astype(bf16)
    P["pwab"] = pwab
    P["bconst"] = bconst

    x = np.asarray(x, np.float32).reshape(B, C, PIX)
    cp = np.asarray(cPromt, np.float32).reshape(B, C, PIX).astype(bf16)
    rp = np.asarray(rPromt, np.float32).reshape(B, C, PIX).astype(bf16)

    per_core = []
    for c in range(N_CORES):
        idx = [c + N_CORES * k for k in range(SPB)]
        m = dict(P)
        m["x_in"] = np.ascontiguousarray(x[idx])
        m["cp_in"] = np.ascontiguousarray(cp[idx])
        m["rp_in"] = np.ascontiguousarray(rp[idx])
        m["thr"] = (thr[idx] - bd).reshape(1, SPB).astype(np.float32)
        per_core.append(m)
    return per_core


_NC_CACHE = {}


def kernel(x, cPromt, rPromt, params):
    from concourse.bass_utils import run_bass_kernel_spmd

    per_core = prep_inputs(x, cPromt, rPromt, params)
    if "nc" not in _NC_CACHE:
        _NC_CACHE["nc"] = build_nc()
    nc = _NC_CACHE["nc"]
    res = run_bass_kernel_spmd(nc, per_core, list(range(N_CORES)))
    out = np.zeros((B, C, PIX), np.float32)
    for c in range(N_CORES):
        oc = res.results[c]["out"]
        for k in range(SPB):
            out[c + N_CORES * k] = oc[k]
    return out.reshape(B, C, H, W)


# revision 28
# speedup vs baseline: 2163.8157x; 1.0091x over previous
"""Trainium2 Bass kernel for nn_D3Net (moe_routing).

Data-parallel over 8 NeuronCores: core c takes samples [c, c+8, c+16, c+24].
Per sample: decision network -> gate; ADB branch (2x gnconv with depthwise
7x7); gated residual combine.

Layout: channels on SBUF partitions, pixels (y-major) on the free dim.
Dense convs/matmuls in bf16 (PE, 1 cyc/row); LN statistics in fp32r.
Depthwise 7x7 = 49 scalar_tensor_tensor MACs per channel group:
low group (abc 0..119, partitions 8..127) on VectorE, high group
(abc 120..247) on GpSimd, in bf16.

`kernel(**inputs)` is the host entry: preps weight layouts, compiles once,
runs SPMD on cores 0-7 via run_bass_kernel_spmd, reassembles full output.
"""

import sys

sys.path.insert(0, "/opt/trn_rl_repo")

import numpy as np

N_CORES = 8
B, C, H, W = 32, 128, 64, 64
PIX = H * W            # 4096
SPB = B // N_CORES     # 4 samples per core

# padded abc layout for depthwise 7x7: 70 rows x 72 cols, interior at (3, 3)
DW_W = 72
DW_F = 70 * DW_W       # 5040
DW_BASE = 3 * DW_W + 3

# dn stages: (in_ch, out_ch, in_side); padded input tile: (side+2) x (side+2)
DN_STAGES = [(128, 64, 64), (64, 32, 32), (32, 16, 16), (16, 8, 8)]

FB = 512    # f-chunk for matmul moving operands (ISA limit)
NB = PIX // FB  # 8
FS = 512    # f-chunk for fp32r LN statistic matmuls
NS = PIX // FS  # 8
COMP_K = [8, 128, 128]


def build_nc():
    from contextlib import ExitStack

    import concourse.bacc as bacc
    import concourse.mybir as mybir
    from concourse import tile

    dt = mybir.dt
    Alu = mybir.AluOpType
    Act = mybir.ActivationFunctionType
    f32r = dt.float32r

    nc = bacc.Bacc("TRN2", target_bir_lowering=False, debug=False,
                   num_devices=N_CORES)

    def din(name, shape, d=dt.float32):
        return nc.dram_tensor(name, shape, d, kind="ExternalInput")

    x_in = din("x_in", (SPB, C, PIX))
    rp_in = din("rp_in", (SPB, C, PIX), dt.bfloat16)
    cp_in = din("cp_in", (SPB, C, PIX), dt.bfloat16)
    out_d = nc.dram_tensor("out", (SPB, C, PIX), dt.float32, kind="ExternalOutput")

    dn_w = [din(f"dn_w{i}", (ci, 9 * co), dt.bfloat16) for i, (ci, co, _) in enumerate(DN_STAGES)]
    dn_b = [din(f"dn_b{i}", (co, 1)) for i, (_, co, _) in enumerate(DN_STAGES)]
    dn4_wd = din("dn4_wd", (8, 1), dt.bfloat16)
    thr_in = din("thr", (1, SPB))        # thr - bd, pre-adjusted on host
    rs_in = din("rs", (1, 1))            # res_scale
    ln1_gb = din("ln1_gb", (C, 2))
    eps_in = din("eps", (8, 1))
    ones_bf = din("ones_bf", (C, 8), dt.bfloat16)
    onehot = din("onehot", (8, C), dt.bfloat16)  # bcast lhsT: row-0 selector
    pi_w = din("pi_w", (C, 8 * C), dt.bfloat16)   # [(ci%128), (br, kg, mg, co)]
    pi_b = din("pi_b", (C, 4))                    # [(co%128), (br, mg)]
    dww = din("dww", (C, 4 * 49))                 # [(part), (br, grp, tap)]
    dwb0 = din("dwb0", (C, 2))                    # rows 8:16 = dw bias 0:8
    comp_w = din("comp_w", (C, 6 * C), dt.bfloat16)   # [(k), (br, term{y1,lo,hi}, co)]
    pwa_w = din("pwa_w", (C, 2 * 2 * 8), dt.bfloat16)  # [(ci), (br, kg, 8)]
    pwab = din("pwab", (8, 2))                         # pwa bias [8, br]
    bconst = din("bconst", (C, 2))
    dwdiag = din("dwdiag", (C, 98 * C), dt.bfloat16)  # PE-tap diags, packed
    beta_g = din("beta_g", (C, 2))
    c11_wT = din("c11_wT", (C, C), dt.bfloat16)
    c11_b = din("c11_b", (C, 1))

    ctx = ExitStack()
    with tile.TileContext(nc, pool_alloc_mode="queue") as tc:
        wp = ctx.enter_context(tc.tile_pool(name="wpool", bufs=1))
        pp = ctx.enter_context(tc.tile_pool(name="perm", bufs=1))
        dpool = ctx.enter_context(tc.tile_pool(name="data", bufs=1))
        db2 = ctx.enter_context(tc.tile_pool(name="dbuf", bufs=2))
        ps = ctx.enter_context(tc.tile_pool(name="psum", bufs=2, space="PSUM"))
        ps4 = ctx.enter_context(tc.tile_pool(name="psum4", bufs=2, space="PSUM"))
        ps1 = ctx.enter_context(tc.tile_pool(name="psum1", bufs=1, space="PSUM"))

        def wtile(src, shape):
            t = wp.tile(list(shape),
                        src.tensor.dtype if hasattr(src, "tensor") else src.dtype,
                        tag=f"w_{src.name}")
            nc.sync.dma_start(out=t[:], in_=src.ap())
            return t

        w_dn = [wtile(dn_w[i], (DN_STAGES[i][0], 9 * DN_STAGES[i][1])) for i in range(4)]
        b_dn = [wtile(dn_b[i], (DN_STAGES[i][1], 1)) for i in range(4)]
        w_d4 = wtile(dn4_wd, (8, 1))
        t_thr = wtile(thr_in, (1, SPB))
        t_rs = wtile(rs_in, (1, 1))
        t_ln1 = wtile(ln1_gb, (C, 2))
        t_eps = wtile(eps_in, (8, 1))
        t_onb = wtile(ones_bf, (C, 8))
        t_oh = wtile(onehot, (8, C))
        t_piw = wtile(pi_w, (C, 8 * C))
        t_pib = wtile(pi_b, (C, 4))
        t_dww = wtile(dww, (C, 4 * 49))
        t_dwb0 = wtile(dwb0, (C, 2))
        t_comp = wtile(comp_w, (C, 6 * C))
        t_pwaw = wtile(pwa_w, (C, 2 * 2 * 8))
        t_pwab = wtile(pwab, (8, 2))
        t_bc = wtile(bconst, (C, 2))
        t_bg = wtile(beta_g, (C, 2))
        t_c11 = wtile(c11_wT, (C, C))
        t_c11b = wtile(c11_b, (C, 1))

        # persistent padded tiles; zero once, interiors rewritten per sample
        dn_pads = []
        for si, (ci, _, side) in enumerate(DN_STAGES):
            t = pp.tile([ci, (side + 2) * (side + 2)], dt.bfloat16, tag=f"dnpad{si}")
            nc.vector.memset(t[:], 0.0)
            dn_pads.append(t)
        flo_p = pp.tile([C, DW_F], dt.bfloat16, tag="flo")
        fhi_p = pp.tile([C, DW_F], dt.bfloat16, tag="fhi")
        flo_o = pp.tile([C, DW_F], dt.bfloat16, tag="floo")
        nc.vector.memset(flo_p[:], 0.0)
        nc.vector.memset(fhi_p[:], 0.0)
        nc.vector.memset(flo_o[:], 0.0)

        def win(ap_2d, rows, stride, r0, nr, c0, ncol):
            """3D window view of a flat [P, rows*stride] AP."""
            v = ap_2d.rearrange("p (r c) -> p r c", r=rows, c=stride)
            return v[:, r0:r0 + nr, c0:c0 + ncol]

        def ln_norm(src, src_is_f32, out_t, gb):
            """out = (src - mu(ch)) * rstd(ch) [* g + b]; src [C, PIX] tile.

            Per 512-pixel chunk: matmul with [C, 8] ones/C -> psum [8, FS]
            (8 identical rows, base partition 0), compact rsqrt via
            exp(-0.5*ln(var+eps)), broadcast back to 128 partitions via a
            one-hot-row matmul, then normalize with two DVE passes.
            """
            xsq = dpool.tile([C, PIX], dt.bfloat16, tag="scratch")
            nc.scalar.square(xsq[:], src[:])
            if src_is_f32:
                src_bf = dpool.tile([C, PIX], dt.bfloat16, tag="srcbf")
                nc.vector.tensor_copy(src_bf[:], src[:])
            else:
                src_bf = src
            for j in range(NS):
                sl = slice(j * FS, (j + 1) * FS)
                stats = ps1.tile([8, 2 * FS], dt.float32, tag="auxs")
                nc.tensor.matmul(stats[:, 0:FS], t_onb[:],
                                 src_bf[:, sl], start=True, stop=True)
                nc.tensor.matmul(stats[:, FS:2 * FS], t_onb[:],
                                 xsq[:, sl], start=True, stop=True)
                musq = dpool.tile([8, FS], dt.float32, tag="musq")
                nc.scalar.square(musq[:], stats[:, 0:FS])
                var_c = dpool.tile([8, FS], dt.float32, tag="varc")
                nc.vector.scalar_tensor_tensor(var_c[:], musq[:], -1.0,
                                               stats[:, FS:2 * FS], Alu.mult, Alu.add)
                lnv = dpool.tile([8, FS], dt.float32, tag="lnv")
                nc.scalar.activation(lnv[:], var_c[:], Act.Ln, bias=t_eps[:])
                rstd_c = dpool.tile([8, FS], dt.bfloat16, tag="rstdc")
                nc.scalar.activation(rstd_c[:], lnv[:], Act.Exp, scale=-0.5)
                murstd_c = dpool.tile([8, FS], dt.bfloat16, tag="murstdc")
                nc.vector.tensor_mul(murstd_c[:], stats[:, 0:FS], rstd_c[:])
                bco = ps1.tile([C, FS], dt.float32, tag="auxb")
                nc.tensor.matmul(bco[:], t_oh[:], rstd_c[:], start=True, stop=True)
                tmp = dpool.tile([C, FS], dt.float32, tag="lntmp")
                nc.vector.tensor_mul(tmp[:], src[:, sl], bco[:])
                bco2 = ps1.tile([C, FS], dt.float32, tag="auxb")
                nc.tensor.matmul(bco2[:], t_oh[:], murstd_c[:], start=True, stop=True)
                if gb is None:
                    nc.vector.tensor_sub(out_t[:, sl], tmp[:], bco2[:])
                else:
                    t2 = dpool.tile([C, FS], dt.float32, tag="lntmp2")
                    nc.vector.tensor_sub(t2[:], tmp[:], bco2[:])
                    nc.vector.tensor_scalar(out_t[:, sl], t2[:], gb[:, 0:1],
                                            gb[:, 1:2], Alu.mult, Alu.add)

        def gnconv(br, xin_bf, dp_bf, h_out):
            """h_out (bf16) = leaky_relu(gnconv_br(xin, dp), 0.01)."""
            # proj_in: two 128-row m-groups; lo rows = [abc 0:120 | pwa 0:8],
            # hi rows = abc 120:248. Bias added during the psum->padded copy.
            for mg, dst in ((0, flo_p), (1, fhi_p)):
                for j in range(NB):
                    pt = ps.tile([C, FB], dt.float32, tag="mm")
                    sl = slice(j * FB, (j + 1) * FB)
                    w0 = t_piw[:, (br * 4 + 0 * 2 + mg) * C:(br * 4 + 0 * 2 + mg + 1) * C]
                    w1 = t_piw[:, (br * 4 + 1 * 2 + mg) * C:(br * 4 + 1 * 2 + mg + 1) * C]
                    nc.tensor.matmul(pt[:], w0, xin_bf[:, sl], start=True, stop=False)
                    nc.tensor.matmul(pt[:], w1, dp_bf[:, sl], start=False, stop=True)
                    rows = FB // W  # 8
                    y0 = j * rows
                    dst_ap = win(dst[:], 70, DW_W, 3 + y0, rows, 3, W)
                    nc.scalar.activation(dst_ap, win(pt[:], rows, W, 0, rows, 0, W),
                                         Act.Identity,
                                         bias=t_pib[:, br * 2 + mg:br * 2 + mg + 1])
            # shifted-by-one copy of flo so odd-dx taps read 4B-aligned (2x mode)
            nc.scalar.activation(flo_o[:, 0:DW_F - 1], flo_p[:, 1:DW_F], Act.Copy)
            # pwa again at base partition 0 (y1 needs it aligned with acc_lo[0:8])
            pwa_t = dpool.tile([8, PIX], dt.bfloat16, tag="pwa")
            for j in range(NB):
                pt = ps.tile([8, FB], dt.float32, tag="mm")
                sl = slice(j * FB, (j + 1) * FB)
                nc.tensor.matmul(pt[:], t_pwaw[:, (br * 2 + 0) * 8:(br * 2 + 1) * 8],
                                 xin_bf[:, sl], start=True, stop=False)
                nc.tensor.matmul(pt[:], t_pwaw[:, (br * 2 + 1) * 8:(br * 2 + 2) * 8],
                                 dp_bf[:, sl], start=False, stop=True)
                nc.scalar.activation(pwa_t[:, sl], pt[:], Act.Identity,
                                     bias=t_pwab[:, br:br + 1])
            # depthwise 7x7 MACs: full 128 partitions (pwa rows have zero taps)
            acc_lo = dpool.tile([C, PIX], dt.bfloat16, tag="acclo")
            acc_hi = dpool.tile([C, PIX], dt.bfloat16, tag="acchi")
            # tap split: DVE gets 2x-eligible even-dx taps, PE (diag matmuls)
            # gets the rest; PE partials merge into acc via in-place TT-adds.
            DVE_TAPS = {0: list(range(49)), 1: []}
            for grp, (srcp, acc) in ((0, (flo_p, acc_lo)), (1, (fhi_p, acc_hi))):
                av = win(acc[:], H, W, 0, H, 0, W)
                dtap = DVE_TAPS[grp]
                ptap = [k for k in range(49) if k not in dtap]
                for i, k in enumerate(dtap):
                    dy, dx = k // 7, k % 7
                    if grp == 0 and dx % 2 == 1:
                        s = win(flo_o[:], 70, DW_W, dy, H, dx - 1, W)
                    else:
                        s = win(srcp[:], 70, DW_W, dy, H, dx, W)
                    wv = t_dww[:, (br * 2 + grp) * 49 + k:(br * 2 + grp) * 49 + k + 1]
                    if i == 0:
                        nc.vector.tensor_scalar(av, s, wv, None, Alu.mult)
                    else:
                        nc.vector.scalar_tensor_tensor(av, s, wv, av, Alu.mult, Alu.add)
                if not ptap:
                    continue
                base = (br * 49 + 0) * C
                dwd_t = dpool.tile([C, len(ptap) * C], dt.bfloat16, tag="dwd",
                                   name=f"dwd{br}_{grp}")
                nc.sync.dma_start(out=dwd_t[:],
                                  in_=dwdiag.ap()[:, base:base + len(ptap) * C])
                for half in range(4):
                    pts = [ps4.tile([C, FB], dt.float32, tag="dwh",
                                    name=f"dwh{br}_{grp}_{half}_{jj}") for jj in range(2)]
                    for i, k in enumerate(ptap):
                        dy, dx = k // 7, k % 7
                        wv = dwd_t[:, i * C:(i + 1) * C]
                        for jj in range(2):
                            j = half * 2 + jj
                            rows = FB // W  # 8
                            s = win(srcp[:], 70, DW_W, dy + j * rows, rows, dx, W)
                            nc.tensor.matmul(pts[jj][:], wv, s,
                                             start=(i == 0), stop=(i == len(ptap) - 1))
                    for jj in range(2):
                        j = half * 2 + jj
                        sl = slice(j * FB, (j + 1) * FB)
                        if dtap:
                            nc.vector.tensor_add(acc[:, sl], acc[:, sl], pts[jj][:])
                        else:
                            nc.scalar.activation(acc[:, sl], pts[jj][:], Act.Copy)
            # y1 = pwa * (dw0 + b0): all operands on partitions 0..7
            y1 = dpool.tile([8, PIX], dt.bfloat16, tag="scratch")
            nc.vector.scalar_tensor_tensor(y1[:], acc_lo[0:8, :],
                                           t_dwb0[0:8, br:br + 1], pwa_t[:],
                                           Alu.add, Alu.mult)
            # composed matmuls (+ bconst, leaky) -> h_out
            rhs_list = [y1[0:8, :], acc_lo[:, :], acc_hi[:, :]]
            for j in range(NB):
                pt = ps.tile([C, FB], dt.float32, tag="mm")
                sl = slice(j * FB, (j + 1) * FB)
                for ti in range(3):
                    kk = COMP_K[ti]
                    wv = t_comp[0:kk, (br * 3 + ti) * C:(br * 3 + ti + 1) * C]
                    nc.tensor.matmul(pt[:], wv, rhs_list[ti][:, sl],
                                     start=(ti == 0), stop=(ti == 2))
                nc.scalar.activation(h_out[:, sl], pt[:], Act.Lrelu,
                                     bias=t_bc[:, br:br + 1], alpha=0.01)

        # ================= per-sample loop =================
        for s in range(SPB):
            xs = dpool.tile([C, PIX], dt.float32, tag="xs")
            nc.sync.dma_start(out=xs[:], in_=x_in.ap()[s])
            rps = dpool.tile([C, PIX], dt.bfloat16, tag="rps")
            nc.sync.dma_start(out=rps[:], in_=rp_in.ap()[s])
            dps = dpool.tile([C, PIX], dt.bfloat16, tag="dps")
            nc.sync.dma_start(out=dps[:], in_=cp_in.ap()[s])


            # ---- ADB ----
            inputs_t = dpool.tile([C, PIX], dt.bfloat16, tag="inputs")
            ln_norm(xs, True, inputs_t, t_ln1)
            h_t = dpool.tile([C, PIX], dt.bfloat16, tag="h")
            gnconv(0, inputs_t, dps, h_t)
            y_t = dpool.tile([C, PIX], dt.bfloat16, tag="y")
            nc.vector.scalar_tensor_tensor(y_t[:], h_t[:], t_bg[:, 0:1], inputs_t[:],
                                           Alu.mult, Alu.add)
            n2_t = dpool.tile([C, PIX], dt.bfloat16, tag="pwa")
            ln_norm(h_t, False, n2_t, None)
            h2a_t = dpool.tile([C, PIX], dt.bfloat16, tag="h2a")
            for j in range(NB):
                pt = ps.tile([C, FB], dt.float32, tag="mm")
                sl = slice(j * FB, (j + 1) * FB)
                nc.tensor.matmul(pt[:], t_c11[:], n2_t[:, sl], start=True, stop=True)
                nc.scalar.activation(h2a_t[:, sl], pt[:], Act.Identity, bias=t_c11b[:])
            h2_t = dpool.tile([C, PIX], dt.bfloat16, tag="inputs")
            gnconv(1, h2a_t, dps, h2_t)
            # ---- decision network ----
            d0 = dn_pads[0]
            din_ap = win(d0[:], 66, 66, 1, H, 1, W)
            nc.vector.tensor_add(din_ap, win(xs[:], H, W, 0, H, 0, W),
                                 win(rps[:], H, W, 0, H, 0, W))
            m2t = None
            for st, (ci, co, side) in enumerate(DN_STAGES):
                n_f = side * side
                fch = min(FB, n_f)
                nchk = n_f // fch
                rows = side // nchk
                pw_ = side + 2
                cur = dn_pads[st]
                scr = dpool.tile([co, n_f], dt.bfloat16, tag="scratch")
                for j in range(nchk):
                    pt = ps.tile([co, fch], dt.float32, tag="mm")
                    for t9 in range(9):
                        dy, dxx = t9 // 3, t9 % 3
                        src = win(cur[0:ci, :], pw_, pw_, dy + j * rows, rows, dxx, side)
                        nc.tensor.matmul(pt[:], w_dn[st][:, t9 * co:(t9 + 1) * co],
                                         src, start=(t9 == 0), stop=(t9 == 8))
                    nc.scalar.activation(scr[:, j * fch:(j + 1) * fch], pt[:],
                                         Act.Relu, bias=b_dn[st])
                hs = side // 2
                m1 = dpool.tile([co, side * hs], dt.bfloat16, tag="dnm1")
                sv = scr[:].rearrange("p (r c2 two) -> p r c2 two", r=side, c2=hs, two=2)
                nc.vector.tensor_max(win(m1[:], side, hs, 0, side, 0, hs),
                                     sv[:, :, :, 0], sv[:, :, :, 1])
                mv = m1[:].rearrange("p (r2 two c) -> p r2 two c", r2=hs, two=2, c=hs)
                if st < 3:
                    nxt = dn_pads[st + 1]
                    pwn = DN_STAGES[st + 1][2] + 2
                    dst = win(nxt[0:co, :], pwn, pwn, 1, hs, 1, hs)
                    nc.vector.tensor_max(dst, mv[:, :, 0, :], mv[:, :, 1, :])
                else:
                    m2t = dpool.tile([8, 16], dt.float32, tag="dnm2")
                    nc.vector.tensor_max(win(m2t[:], 4, 4, 0, 4, 0, 4),
                                         mv[:, :, 0, :], mv[:, :, 1, :])
            hmean = dpool.tile([8, 1], dt.float32, tag="hmean")
            nc.vector.tensor_reduce(hmean[:], m2t[:], mybir.AxisListType.X, Alu.add)
            hmean_bf = dpool.tile([8, 1], dt.bfloat16, tag="hmeanbf")
            nc.vector.tensor_scalar(hmean_bf[:], hmean[:], 1.0 / 16.0, None, Alu.mult)
            ldp = ps.tile([1, 1], dt.float32, tag="mm")
            nc.tensor.matmul(ldp[:], w_d4[:], hmean_bf[:], start=True, stop=True)
            # sdec = (ldiff > thr') * rs
            sdec = dpool.tile([1, 1], dt.float32, tag="sdec")
            nc.vector.tensor_scalar(sdec[:], ldp[:], t_thr[0:1, s:s + 1], t_rs[:],
                                    Alu.is_gt, Alu.mult)
            sdec_b = dpool.tile([C, 1], dt.float32, tag="sdecb")
            nc.gpsimd.partition_broadcast(sdec_b[:], sdec[:])

            a_t = dpool.tile([C, PIX], dt.bfloat16, tag="A")
            nc.vector.scalar_tensor_tensor(a_t[:], h2_t[:], t_bg[:, 1:2], y_t[:],
                                           Alu.mult, Alu.add)
            o_t = dpool.tile([C, PIX], dt.float32, tag="o")
            nc.sync.dma_start(out=o_t[:], in_=x_in.ap()[s])
            nc.vector.scalar_tensor_tensor(o_t[:], a_t[:], sdec_b[:], o_t[:],
                                           Alu.mult, Alu.add)
            nc.sync.dma_start(out=out_d.ap()[s], in_=o_t[:])

        ctx.close()

    nc.compile()
    return nc


# ------------------------------------------------------------------
def prep_inputs(x, cPromt, rPromt, params):
    import jax
    import ml_dtypes

    bf16 = ml_dtypes.bfloat16
    g = lambda a: np.asarray(a, np.float32)

    P = {}
    for i, (ci, co, _) in enumerate(DN_STAGES):
        w, b = params["dn"][i]
        w = g(w)
        lhs = np.zeros((ci, 9 * co), np.float32)
        for t in range(9):
            lhs[:, t * co:(t + 1) * co] = w[:, :, t // 3, t % 3].T
        P[f"dn_w{i}"] = lhs.astype(bf16)
        P[f"dn_b{i}"] = g(b).reshape(-1, 1)
    w4, b4 = params["dn"][4]
    w4 = g(w4)[:, :, 0, 0]
    P["dn4_wd"] = (w4[1] - w4[0]).reshape(8, 1).astype(bf16)
    bd = float(g(b4)[1] - g(b4)[0])

    # The reference's gumbel noise: jax threefry streams differ between the
    # axon-neuron backend and CPU. The harness's reference runs on CPU (the
    # model cannot jit-compile on neuron), so pin the CPU stream explicitly.
    with jax.default_device(jax.devices("cpu")[0]):
        u = np.asarray(jax.random.uniform(jax.random.key(7), (B, 2), np.float32,
                                          1e-6, 1.0 - 1e-6), np.float64)
    gum = -np.log(-np.log(u))
    G = gum[:, 0] - gum[:, 1]
    thr = np.where(np.abs(G) < 1.0,
                   2.0 * np.arctanh(np.clip(G, -1 + 1e-12, 1 - 1e-12)),
                   np.where(G >= 1.0, 1e30, -1e30))
    P["rs"] = np.full((1, 1), float(np.asarray(params["res_scale"])), np.float32)

    adb = params["adb"]
    P["ln1_gb"] = np.stack([g(adb["ln1"][0]), g(adb["ln1"][1])], 1).astype(np.float32)
    P["eps"] = np.full((8, 1), 1e-6, np.float32)
    P["ones_bf"] = np.full((C, 8), 1.0 / C, np.float32).astype(bf16)
    oh = np.zeros((8, C), np.float32)
    oh[0, :] = 1.0
    P["onehot"] = oh.astype(bf16)
    ln2_g, ln2_b = g(adb["ln2"][0]), g(adb["ln2"][1])
    W11 = g(adb["conv11"][0])[:, :, 0, 0]
    P["c11_wT"] = (W11 * ln2_g[None, :]).T.astype(bf16)
    P["c11_b"] = (g(adb["conv11"][1]) + W11 @ ln2_b).reshape(-1, 1).astype(np.float32)
    P["beta_g"] = np.stack([g(adb["beta"]).reshape(C), g(adb["gamma"]).reshape(C)],
                           1).astype(np.float32)

    pi_w = np.zeros((C, 8 * C), np.float32)
    pi_b = np.zeros((C, 4), np.float32)
    dww = np.zeros((C, 4 * 49), np.float32)
    dwb0 = np.zeros((C, 2), np.float32)
    comp_w = np.zeros((C, 6 * C), np.float32)
    dwdiag_h = np.zeros((C, 98 * C), np.float32)
    pwa_w = np.zeros((C, 2 * 2 * 8), np.float32)
    pwab = np.zeros((8, 2), np.float32)
    bconst = np.zeros((C, 2), np.float32)
    # lo m-group row r: abc channel r (fused 8+r) for r<120, pwa r-120 for r>=120
    perm_lo = np.array([8 + r for r in range(120)] + [r for r in range(8)])
    for bi, name in enumerate(("b1", "b2")):
        q = adb[name]
        Wpi = g(q["proj_in_mf"][0])[:, :, 0, 0]
        bpi = g(q["proj_in_mf"][1])
        for kg in range(2):
            blk = Wpi[:, kg * C:(kg + 1) * C]
            for mg in range(2):
                col = (bi * 4 + kg * 2 + mg) * C
                rows = perm_lo if mg == 0 else np.arange(C, 2 * C)
                pi_w[:, col:col + C] = blk[rows, :].T
            pwa_w[:, (bi * 2 + kg) * 8:(bi * 2 + kg + 1) * 8] = blk[0:8, :].T
        pi_b[:, bi * 2 + 0] = bpi[perm_lo]
        pi_b[:, bi * 2 + 1] = bpi[C:2 * C]
        pwab[:, bi] = bpi[0:8]
        wdw = g(q["dwconv"][0])[:, 0].reshape(248, 49)
        dww[0:120, (bi * 2 + 0) * 49:(bi * 2 + 1) * 49] = wdw[0:120]
        dww[:, (bi * 2 + 1) * 49:(bi * 2 + 2) * 49] = wdw[120:248]
        for i in range(49):
            col = (bi * 49 + i) * C
            np.fill_diagonal(dwdiag_h[:, col:col + C], wdw[120:248, i])
        bdw = g(q["dwconv"][1])
        dwb0[0:8, bi] = bdw[0:8]
        pw = [g(q["pws"][i][0])[:, :, 0, 0] for i in range(4)]
        pwb = [g(q["pws"][i][1]) for i in range(4)]
        Wpo = g(q["proj_out"][0])[:, :, 0, 0]
        bpo = g(q["proj_out"][1])
        C4 = Wpo @ pw[3]; C3 = C4 @ pw[2]; C2 = C3 @ pw[1]; C1 = C2 @ pw[0]
        bc = (Wpo @ pwb[3] + C4 @ pwb[2] + C3 @ pwb[1] + C2 @ pwb[0] + bpo
              + C2 @ bdw[8:24] + C3 @ bdw[24:56] + C4 @ bdw[56:120]
              + Wpo @ bdw[120:248])
        # term 0: y1 (K=8); term 1: merged lo (K=128, rows=acc_lo channels);
        # term 2: hi (K=128)
        comp_w[0:8, (bi * 3 + 0) * C:(bi * 3 + 1) * C] = C1.T
        merged = np.zeros((C, C), np.float32)
        merged[8:24, :] = C2.T       # dw1 = abc 8:24 at partitions 8:24
        merged[24:56, :] = C3.T
        merged[56:120, :] = C4.T
        comp_w[:, (bi * 3 + 1) * C:(bi * 3 + 2) * C] = merged
        comp_w[:, (bi * 3 + 2) * C:(bi * 3 + 3) * C] = Wpo.T
        bconst[:, bi] = bc
    P["pi_w"] = pi_w.astype(bf16)
    P["pi_w"] = pi_w.astype(bf16)
    P["pi_b"] = pi_b
    P["dww"] = dww
    P["dwb0"] = dwb0
    P["comp_w"] = comp_w.astype(bf16)
    P["dwdiag"] = dwdiag_h.astype(bf16)
    P["pwa_w"] = pwa_w.astype(bf16)
    P["pwab"] = pwab
    P["bconst"] = bconst

    x = np.asarray(x, np.float32).reshape(B, C, PIX)
    cp = np.asarray(cPromt, np.float32).reshape(B, C, PIX).astype(bf16)
    rp = np.asarray(rPromt, np.float32).reshape(B, C, PIX).astype(bf16)

    per_core = []
    for c in range(N_CORES):
        idx = [c + N_CORES * k for k in range(SPB)]
        m = dict(P)
        m["x_in"] = np.ascontiguousarray(x[idx])
        m["cp_in"] = np.ascontiguousarray(cp[idx])
        m["rp_in"] = np.ascontiguousarray(rp[idx])
        m["thr"] = (thr[idx] - bd).reshape(1, SPB).astype(np.float32)
        per_core.append(m)
    return per_core


_NC_CACHE = {}


def kernel(x, cPromt, rPromt, params):
    from concourse.bass_utils import run_bass_kernel_spmd

    per_core = prep_inputs(x, cPromt, rPromt, params)
    if "nc" not in _NC_CACHE:
        _NC_CACHE["nc"] = build_nc()
    nc = _NC_CACHE["nc"]
    res = run_bass_kernel_spmd(nc, per_core, list(range(N_CORES)))
    out = np.zeros((B, C, PIX), np.float32)
    for c in range(N_CORES):
        oc = res.results[c]["out"]
        for k in range(SPB):
            out[c + N_CORES * k] = oc[k]
    return out.reshape(B, C, H, W)
